# revision 29
# baseline (speedup 1.0000x reference)
"""Multi-head attention (projections + softmax(QK^T/sqrt(d)) @ V) for Trainium2.

Sharding: 32 (batch, head) pairs split across 8 NeuronCores -> 4 heads/core.
Core c gets batch b = c // 4 and heads h0 = (c % 4) * 4 .. h0+4, so each
core's input slice query[b, :, h0:h0+4, :] has contiguous 2KB DMA lines.

Per-head pipeline on each core:
  1. PE-transpose raw q/k/v 128x128 tiles (float32r transpose-mode, 1.5
     cyc/row; identity supplied as a host input since fp32r operands must be
     DMA-produced) -> [e_in, s] layout, cast to fp16 in the PSUM->SBUF copy.
  2. Project with fp16 matmuls (1 cyc/row): qT/kT = WT.T @ rawT (N=512),
     bias-add fused into the PSUM->SBUF copy, fp16 out in per-512-block
     tiles (block-granular deps).  v is projected back to natural [s, e]
     layout (lhsT = rawT chunk, rhs = WvT) and stored fp16 with a column of
     ones appended (col 128) for the softmax row sums.
  3. Attention, software-pipelined per 512-wide i-tile: each super of 2
     j-chunks runs [2 scoresT matmuls (fp16, N=512, fp32 PSUM) -> one
     1024-wide exp on ScalarE (1/sqrt(d) scale fused, unnormalized, fp16
     out; scores in [-11, 11] so exp fits fp16) -> 8 AV matmuls], keeping
     ScalarE (the 2nd-busiest engine) saturated while PE runs AV.  The 4 AV
     accumulation chains pack two-per-PSUM-bank: a zero "opener" matmul
     (start=True) clears each bank's has_written bits, then both chains
     accumulate with start=False under per-element has_written semantics.
     out[i, 0:129] += expT_chunk.T @ v_ext accumulates the weighted values
     and the row sums together; normalize with DVE reciprocal + multiply.

Engine budget per core (cost model): PE 139us, ACT 134us, DVE 81us;
modeled exec ~193us single-shot / ~149us steady-state.
"""

import math
import os
import sys

import numpy as np

for _p in ("/opt/trn_rl_repo",):
    if _p not in sys.path and os.path.isdir(_p):
        sys.path.insert(0, _p)

B, S, H, E = 2, 2048, 16, 128
N_CORES = 8
HPC = (B * H) // N_CORES  # heads per core = 4
P = 128
NSC = S // P  # seq chunks of 128 = 16
NT = S // 512  # 512-wide seq tiles = 4
SCALE = 1.0 / math.sqrt(E)


def build_bass(reps=1, dma_tpose=False):
    from contextlib import ExitStack

    import concourse.mybir as mybir
    import concourse.tile as tile
    from concourse import bacc

    f32 = mybir.dt.float32
    f32r = mybir.dt.float32r
    f16 = mybir.dt.float16
    Exp = mybir.ActivationFunctionType.Exp

    nc = bacc.Bacc()
    q4 = nc.dram_tensor("q4", [S, HPC, E], f32, kind="ExternalInput").ap()
    k4 = nc.dram_tensor("k4", [S, HPC, E], f32, kind="ExternalInput").ap()
    v4 = nc.dram_tensor("v4", [S, HPC, E], f32, kind="ExternalInput").ap()
    Wq = nc.dram_tensor("Wq", [E, E], f32, kind="ExternalInput").ap()
    Wk = nc.dram_tensor("Wk", [E, E], f32, kind="ExternalInput").ap()
    Wv = nc.dram_tensor("Wv", [E, E], f32, kind="ExternalInput").ap()
    bq = nc.dram_tensor("bq", [E, 1], f32, kind="ExternalInput").ap()
    bk = nc.dram_tensor("bk", [E, 1], f32, kind="ExternalInput").ap()
    bv = nc.dram_tensor("bv", [1, E], f32, kind="ExternalInput").ap()
    identity = nc.dram_tensor("identity", [E, E], f32, kind="ExternalInput").ap()
    out4 = nc.dram_tensor("out4", [HPC, S, E], f32, kind="ExternalOutput").ap()

    q4r = q4.rearrange("(c p) h e -> p c h e", p=P)
    k4r = k4.rearrange("(c p) h e -> p c h e", p=P)
    v4r = v4.rearrange("(c p) h e -> p c h e", p=P)

    with tile.TileContext(nc) as tc, ExitStack() as ctx:
        singles = ctx.enter_context(tc.tile_pool(name="singles", bufs=1))
        rawq = ctx.enter_context(tc.tile_pool(name="rawq", bufs=NSC))
        rawk = ctx.enter_context(tc.tile_pool(name="rawk", bufs=NSC))
        rawv = ctx.enter_context(tc.tile_pool(name="rawv", bufs=NSC))
        if dma_tpose:
            r16q = ctx.enter_context(tc.tile_pool(name="r16q", bufs=NSC))
            r16k = ctx.enter_context(tc.tile_pool(name="r16k", bufs=NSC))
            r16v = ctx.enter_context(tc.tile_pool(name="r16v", bufs=NSC))
        stage = ctx.enter_context(tc.tile_pool(name="stage", bufs=2))
        qk16 = ctx.enter_context(tc.tile_pool(name="qk16", bufs=2 * NT))
        vextp = ctx.enter_context(tc.tile_pool(name="vextp", bufs=2 * NT))
        expstore = ctx.enter_context(tc.tile_pool(name="expstore", bufs=3))
        outpool = ctx.enter_context(tc.tile_pool(name="outpool", bufs=4))
        small = ctx.enter_context(tc.tile_pool(name="small", bufs=3))
        # PSUM budget: "sc" 2 slots x 2 banks (scores) + "tp" 2 x 1
        # (ph1 transposes/projections) + "av" 2 x 1 = 8 banks.  Each av bank
        # holds TWO accumulation chains: a zero opener matmul (start=True)
        # clears the bank's has_written bits and zero-fills it, then both
        # chains accumulate with start=False (per-element has_written).
        pbig = ctx.enter_context(tc.tile_pool(name="pbig", bufs=2, space="PSUM"))
        ptp = ctx.enter_context(tc.tile_pool(name="ptp", bufs=2, space="PSUM"))
        pav = ctx.enter_context(tc.tile_pool(name="pav", bufs=2, space="PSUM"))

        ident = singles.tile([P, P], f32r, name="ident")
        nc.sync.dma_start(out=ident[:], in_=identity.bitcast(f32r))

        # Projection weights transposed to [e_in, e_out], cast to fp16
        wT = {}
        for name, w_ap in (("q", Wq), ("k", Wk), ("v", Wv)):
            w_nat = singles.tile([P, P], f32r, name=f"wnat_{name}")
            nc.sync.dma_start(out=w_nat[:], in_=w_ap.bitcast(f32r))
            tpw = ptp.tile([P, 512], f32r, tag="tp", name=f"tpw_{name}")
            nc.tensor.transpose(tpw[:, 0:P], w_nat[:], ident[:])
            wts = singles.tile([P, P], f16, name=f"wT_{name}")
            nc.vector.tensor_copy(out=wts[:], in_=tpw[:, 0:P])
            wT[name] = wts

        zrow = singles.tile([1, 264], f16, name="zrow")
        nc.vector.memset(zrow[:], 0.0)

        bq_sb = singles.tile([P, 1], f32, name="bq_sb")
        nc.gpsimd.dma_start(out=bq_sb[:], in_=bq)
        bk_sb = singles.tile([P, 1], f32, name="bk_sb")
        nc.gpsimd.dma_start(out=bk_sb[:], in_=bk)
        bv_bc = singles.tile([P, E], f32, name="bv_bc")
        nc.gpsimd.dma_start(out=bv_bc[:], in_=bv.to_broadcast((P, E)))

        for rep in range(reps):
          # Bulk input loads, one DMA per 128-row seq chunk (2KB lines)
          qraw, kraw, vraw = [], [], []
          for sc in range(NSC):
            qt = rawq.tile([P, HPC, E], f32r, tag="qraw", name=f"qraw{sc}")
            nc.sync.dma_start(out=qt[:], in_=q4r[:, sc, :, :].bitcast(f32r))
            qraw.append(qt)
            kt = rawk.tile([P, HPC, E], f32r, tag="kraw", name=f"kraw{sc}")
            nc.sync.dma_start(out=kt[:], in_=k4r[:, sc, :, :].bitcast(f32r))
            kraw.append(kt)
            vt = rawv.tile([P, HPC, E], f32r, tag="vraw", name=f"vraw{sc}")
            nc.sync.dma_start(out=vt[:], in_=v4r[:, sc, :, :].bitcast(f32r))
            vraw.append(vt)
          if dma_tpose:
            q16, k16, v16 = [], [], []
            for sc in range(NSC):
                q16t = r16q.tile([P, HPC, E], f16, tag="q16", name=f"q16_{sc}")
                nc.vector.tensor_copy(out=q16t[:], in_=qraw[sc][:])
                q16.append(q16t)
                k16t = r16k.tile([P, HPC, E], f16, tag="k16", name=f"k16_{sc}")
                nc.vector.tensor_copy(out=k16t[:], in_=kraw[sc][:])
                k16.append(k16t)
                v16t = r16v.tile([P, HPC, E], f16, tag="v16", name=f"v16_{sc}")
                nc.vector.tensor_copy(out=v16t[:], in_=vraw[sc][:])
                v16.append(v16t)

          for hl in range(HPC):
            # ---- Phase 1: transposes + projections for head hl ----
            # Per-512-block tiles so phase 2 deps are block-granular: the
            # first scores matmul only waits for its own projection block.
            qT16 = [
                qk16.tile([P, 512], f16, tag="qT16", name=f"qT16_{hl}_{t}")
                for t in range(NT)
            ]
            kT16 = [
                qk16.tile([P, 512], f16, tag="kT16", name=f"kT16_{hl}_{t}")
                for t in range(NT)
            ]
            v_ext = [
                vextp.tile([P, 4, 132], f16, tag="vext", name=f"vext_{hl}_{t}")
                for t in range(NT)
            ]
            for t in range(NT):
                nc.gpsimd.memset(v_ext[t][:, :, 128:129], 1.0)

            for t in range(NT) if not dma_tpose else []:
                # 4 transposes land in one PSUM bank; one 512-wide cast-copy
                qst = stage.tile([P, 512], f16, tag="qst", name=f"qst_{hl}_{t}")
                kst = stage.tile([P, 512], f16, tag="kst", name=f"kst_{hl}_{t}")
                tpq = ptp.tile([P, 512], f32r, tag="tp", name=f"tpq_{hl}_{t}")
                tpk = ptp.tile([P, 512], f32r, tag="tp", name=f"tpk_{hl}_{t}")
                for cc in range(4):
                    sc = t * 4 + cc
                    nc.tensor.transpose(
                        tpq[:, cc * P : (cc + 1) * P], qraw[sc][:, hl, :], ident[:]
                    )
                    nc.tensor.transpose(
                        tpk[:, cc * P : (cc + 1) * P], kraw[sc][:, hl, :], ident[:]
                    )
                nc.vector.tensor_copy(out=qst[:], in_=tpq[:])
                nc.vector.tensor_copy(out=kst[:], in_=tpk[:])
                pq = ptp.tile([P, 512], f32, tag="tp", name=f"pq_{hl}_{t}")
                nc.tensor.matmul(pq[:], wT["q"][:], qst[:], start=True, stop=True)
                nc.vector.tensor_scalar_add(
                    out=qT16[t][:], in0=pq[:], scalar1=bq_sb[:]
                )
                pk = ptp.tile([P, 512], f32, tag="tp", name=f"pk_{hl}_{t}")
                nc.tensor.matmul(pk[:], wT["k"][:], kst[:], start=True, stop=True)
                nc.vector.tensor_scalar_add(
                    out=kT16[t][:], in0=pk[:], scalar1=bk_sb[:]
                )
                # v: transpose 4 chunks, project back to natural [s, e] layout
                tpv = ptp.tile([P, 512], f32r, tag="tp", name=f"tpv_{hl}_{t}")
                for cc in range(4):
                    sc = t * 4 + cc
                    nc.tensor.transpose(
                        tpv[:, cc * P : (cc + 1) * P], vraw[sc][:, hl, :], ident[:]
                    )
                vTc = small.tile([P, 512], f16, tag="vTc", name=f"vTc_{hl}_{t}")
                nc.vector.tensor_copy(out=vTc[:], in_=tpv[:])
                pv = ptp.tile([P, 512], f32, tag="tp", name=f"pv_{hl}_{t}")
                for cc in range(4):
                    nc.tensor.matmul(
                        pv[:, cc * P : (cc + 1) * P],
                        vTc[:, cc * P : (cc + 1) * P],
                        wT["v"][:],
                        start=True,
                        stop=True,
                    )
                nc.vector.tensor_add(
                    out=v_ext[t][:, :, 0:128],
                    in0=pv[:].rearrange("p (c e) -> p c e", c=4),
                    in1=bv_bc[:, None, :].to_broadcast((P, 4, E)),
                )

            if dma_tpose:
              for t in range(NT):
                qst = stage.tile([P, 512], f16, tag="qst", name=f"dqst_{hl}_{t}")
                kst = stage.tile([P, 512], f16, tag="kst", name=f"dkst_{hl}_{t}")
                vTc = small.tile([P, 512], f16, tag="vTc", name=f"dvTc_{hl}_{t}")
                for cc in range(4):
                    sc = t * 4 + cc
                    nc.sync.dma_start_transpose(
                        qst[:, cc * P : (cc + 1) * P], q16[sc][:, hl, :]
                    )
                    nc.sync.dma_start_transpose(
                        kst[:, cc * P : (cc + 1) * P], k16[sc][:, hl, :]
                    )
                    nc.sync.dma_start_transpose(
                        vTc[:, cc * P : (cc + 1) * P], v16[sc][:, hl, :]
                    )
                pq = ptp.tile([P, 512], f32, tag="tp", name=f"dpq_{hl}_{t}")
                nc.tensor.matmul(pq[:], wT["q"][:], qst[:], start=True, stop=True)
                nc.vector.tensor_scalar_add(
                    out=qT16[t][:], in0=pq[:], scalar1=bq_sb[:]
                )
                pk = ptp.tile([P, 512], f32, tag="tp", name=f"dpk_{hl}_{t}")
                nc.tensor.matmul(pk[:], wT["k"][:], kst[:], start=True, stop=True)
                nc.vector.tensor_scalar_add(
                    out=kT16[t][:], in0=pk[:], scalar1=bk_sb[:]
                )
                pv = ptp.tile([P, 512], f32, tag="tp", name=f"dpv_{hl}_{t}")
                for cc in range(4):
                    nc.tensor.matmul(
                        pv[:, cc * P : (cc + 1) * P],
                        vTc[:, cc * P : (cc + 1) * P],
                        wT["v"][:],
                        start=True,
                        stop=True,
                    )
                nc.vector.tensor_add(
                    out=v_ext[t][:, :, 0:128],
                    in0=pv[:].rearrange("p (c e) -> p c e", c=4),
                    in1=bv_bc[:, None, :].to_broadcast((P, 4, E)),
                )

            # ---- Phase 2: attention for head hl ----
            # Software-pipelined per 512-wide i-tile: each super of 2 j-chunks
            # does [2 scoresT matmuls -> one 1024-wide exp -> 8 AV matmuls],
            # so ScalarE stays busy while PE runs AV.  The 4 AV accumulation
            # chains (one per 128-row i-chunk) pack two-per-bank via the zero
            # openers below (start=True clears has_written bank-wide, so
            # chains themselves must run with start=False).
            for it in range(NT):
                avb = [
                    pav.tile([P, 2, 132], f32, tag="av", name=f"av_{hl}_{it}_{b}")
                    for b in range(2)
                ]
                # zero openers: one K=1 matmul per bank clears has_written
                # bank-wide and fills with zeros; chains then accumulate with
                # start=False under per-element has_written semantics.
                for b in range(2):
                    nc.tensor.matmul(
                        avb[b][:, :, :].rearrange("p a b -> p (a b)"),
                        zrow[:, 0:P],
                        zrow[:],
                        start=True,
                        stop=False,
                        skip_group_check=True,
                    )
                av = [
                    avb[0][:, 0, 0:129],
                    avb[0][:, 1, 0:129],
                    avb[1][:, 0, 0:129],
                    avb[1][:, 1, 0:129],
                ]

                def emit_scores(sj):
                    scps = pbig.tile(
                        [P, 1024], f32, tag="sc", name=f"sc_{hl}_{it}_{sj}"
                    )
                    for jj in range(2):
                        jc = sj * 2 + jj
                        nc.tensor.matmul(
                            scps[:, jj * 512 : (jj + 1) * 512],
                            kT16[jc // 4][:, (jc % 4) * P : (jc % 4 + 1) * P],
                            qT16[it][:],
                            start=True,
                            stop=True,
                        )
                    estore = expstore.tile(
                        [P, 1024], f16, tag="estore", name=f"es_{hl}_{it}_{sj}"
                    )
                    nc.scalar.activation(estore[:], scps[:], Exp, scale=SCALE)
                    return estore

                def emit_av(sj, estore):
                    for jj in range(2):
                        jc = sj * 2 + jj
                        for ic in range(4):
                            nc.tensor.matmul(
                                av[ic],
                                estore[
                                    :, jj * 512 + ic * P : jj * 512 + (ic + 1) * P
                                ],
                                v_ext[jc // 4][:, jc % 4, 0:129],
                                start=False,
                                stop=(jc == NSC - 1 and ic % 2 == 1),
                                skip_group_check=True,
                            )

                # one-super software pipeline: emit scores(sj+1) before AV(sj)
                # so PE's in-order stream overlaps AV with the exp on ScalarE
                prev = emit_scores(0)
                for sj in range(1, NSC // 2):
                    cur = emit_scores(sj)
                    emit_av(sj - 1, prev)
                    prev = cur
                emit_av(NSC // 2 - 1, prev)
                for ic in range(4):
                    avt = av[ic]
                    recip = small.tile([P, 1], f32, tag="recip", name=f"rc_{hl}_{it}_{ic}")
                    nc.vector.reciprocal(out=recip[:], in_=avt[:, 128:129])
                    osb = outpool.tile([P, E], f32, tag="osb", name=f"osb_{hl}_{it}_{ic}")
                    nc.vector.tensor_scalar_mul(
                        out=osb[:], in0=avt[:, 0:128], scalar1=recip[:]
                    )
                    base = it * 512 + ic * P
                    nc.sync.dma_start(
                        out=out4[hl, base : base + P, :], in_=osb[:]
                    )

    nc.compile()
    return nc


def _in_maps(inputs):
    query = np.asarray(inputs["query"], dtype=np.float32)
    key = np.asarray(inputs["key"], dtype=np.float32)
    value = np.asarray(inputs["value"], dtype=np.float32)
    Wq = np.ascontiguousarray(np.asarray(inputs["Wq"], dtype=np.float32))
    Wk = np.ascontiguousarray(np.asarray(inputs["Wk"], dtype=np.float32))
    Wv = np.ascontiguousarray(np.asarray(inputs["Wv"], dtype=np.float32))
    bq = np.ascontiguousarray(np.asarray(inputs["bq"], np.float32).reshape(E, 1))
    bk = np.ascontiguousarray(np.asarray(inputs["bk"], np.float32).reshape(E, 1))
    bv = np.ascontiguousarray(np.asarray(inputs["bv"], np.float32).reshape(1, E))
    maps = []
    for c in range(N_CORES):
        b = c // (N_CORES // B)
        h0 = (c % (N_CORES // B)) * HPC
        maps.append(
            {
                "q4": np.ascontiguousarray(query[b, :, h0 : h0 + HPC, :]),
                "k4": np.ascontiguousarray(key[b, :, h0 : h0 + HPC, :]),
                "v4": np.ascontiguousarray(value[b, :, h0 : h0 + HPC, :]),
                "Wq": Wq,
                "Wk": Wk,
                "Wv": Wv,
                "bq": bq,
                "bk": bk,
                "bv": bv,
                "identity": np.eye(E, dtype=np.float32),
            }
        )
    return maps


def run(inputs, trace=False, trace_kwargs=None):
    """Build + run on 8 cores; returns (output, BassKernelResults)."""
    from concourse.bass_utils import run_bass_kernel_spmd

    nc = build_bass()
    res = run_bass_kernel_spmd(
        nc,
        _in_maps(inputs),
        core_ids=list(range(N_CORES)),
        trace=trace,
        **(trace_kwargs or {}),
    )
    out = np.empty((B, H, S, E), dtype=np.float32)
    for c in range(N_CORES):
        b = c // (N_CORES // B)
        h0 = (c % (N_CORES // B)) * HPC
        out[b, h0 : h0 + HPC] = res.results[c]["out4"]
    return out, res


def kernel(**inputs):
    out, _ = run(inputs, trace=False)
    return out
